# revision 27
# baseline (speedup 1.0000x reference)
"""HGRNBitMLPFixed (BitLinear MLP with Q16 fixed-point GLU) on 8 Trainium2 cores.

Strategy (v3)
-------------
Data-parallel over tokens: 8192 tokens -> 1024 per core, weights replicated.
Host pre-quantizes weights (ternary, exact in fp8e4m3; the quant scale is
computed with jax to match the oracle's f32 mean bit-exactly) and packs them;
the device does everything else:

  per token-tile [128, H]:
    stats (sum x^2, absmax) -> r = 1/sqrt(mean+eps), quant scale a1, dequant c1
    q = rne(x * a1)  (int8-valued, exact in bf16)          [natural layout]
    DMA-XBAR transposes -> qT [h, t]
  mm1: psum[t, o] = sum_h qT[h,t]^T wgT[h,o]   (exact integer sums in fp32 PSUM)
  GLU in Q16 fixed point (all exact fp32 integer arithmetic; rne via the
  +/- 1.5*2^23 magic-number trick, floor via rne(x-0.5)):
    gfx = rne(psum_g * c1 * 2^16); sig = sigmoid(gfx/2^16)
    sfx = rne(sig * 2^16); swish = floor(gfx*sfx / 2^16)
    yfx = rne(psum_y * c1 * 2^16); z_fx = floor(swish*yfx / 2^16)
  stage-2 stats over z_fx -> a2, c2; q2 = rne(z_fx * a2); transpose -> q2T
  mm2: psum[t, o] = sum_i q2T^T wdT; out = psum * c2

v3 performance changes vs v2 (identical arithmetic):
  - batched DMA: all 16 h-tiles of one mm1 weight chunk come in ONE
    dma_start (weights repacked in DRAM as [128, 16, NOC, 2, CH]); x loads,
    out stores and XBAR transposes are batched too.  Each dma_start costs
    ~600ns of HWDGE queue time regardless of size, and one large InstDMACopy
    is split across all 16 SDMA engines, so fewer+bigger is strictly better.
  - per pass: 11 wg loads (was 176), 8 transposes (was 30), 4 x loads,
    2 out stores.
  - all transposes on the sync queue (concurrent XBAR transposes from both
    HWDGE queues corrupt each other); big weight loads on the scalar queue.
  - LAG_D 20 -> 14 so wd streaming needs 18 bufs instead of 22 (SBUF).
"""

import numpy as np
import ml_dtypes

import concourse.bass as bass
import concourse.mybir as mybir
import concourse.tile as tile
from concourse import bacc
from concourse.bass_utils import run_bass_kernel_spmd

F32 = mybir.dt.float32
BF16 = mybir.dt.bfloat16
FP8 = mybir.dt.float8e4
AF = mybir.ActivationFunctionType
ALU = mybir.AluOpType
AX = mybir.AxisListType

P = 128          # partitions
CH = 512         # free-dim chunk (one PSUM bank of fp32)
MAGIC = 12582912.0   # 1.5 * 2^23: x + MAGIC - MAGIC == rne(x) for |x| < 2^22
FIX = 65536.0        # 2^16 (Q16)
FIXI = 1.0 / 65536.0
LAG_D = 10       # tt1 chain lag (in s2 steps) in phase D
PF_D = 2         # wd prefetch distance (s2 steps)
WD_BUFS = 17     # > LAG_D + PF_D + 1 (alive span); extra bufs = ring slack


def build_program(T, H, I, wg_deq, wd_deq):
    pass_tt = 2
    NT = T // P               # token tiles (8)
    NH = H // P               # h-tiles (contraction of mm1) (16)
    NHC = H // CH             # h chunks of 512 (4)
    NOC = I // CH             # inter chunks of 512 (11)
    NS2 = I // P              # i-tiles (contraction of mm2) (44)
    NOT = H // CH             # output chunks of 512 (4)
    assert NT % pass_tt == 0
    NPASS = NT // pass_tt
    # C-phase transpose split: s2-tiles [0,20) and [20,44)
    CSPLIT = 20

    nc = bacc.Bacc(None, target_bir_lowering=False)
    x = nc.dram_tensor("x", [T, H], F32, kind="ExternalInput").ap()
    # packed gate|y chunk pairs: [NOC, p, s, 2, CH] fp8 (p=row within h-tile)
    # -> one oc load is 128 contiguous 16KB runs (one DMA descriptor per
    #    partition) instead of 2048 scattered 1KB runs
    wgp = nc.dram_tensor("wgp", [NOC, P, NH, 2, CH], FP8,
                         kind="ExternalInput").ap()
    wdt = nc.dram_tensor("wdt", [I, H], FP8, kind="ExternalInput").ap()
    out = nc.dram_tensor("out", [T, H], F32, kind="ExternalOutput").ap()

    k_c1 = float(wg_deq) * FIX / 127.0   # c1s = mnorm * k_c1 (includes 2^16)
    k_c2 = float(wd_deq) / 127.0         # c2 = mnorm2 * k_c2

    with tile.TileContext(nc) as tc:
        with (
            tc.tile_pool(name="consts", bufs=1) as cpool,
            tc.tile_pool(name="xin", bufs=2) as xin_pool,
            tc.tile_pool(name="persist", bufs=1) as persist,      # z
            tc.tile_pool(name="persist2", bufs=2) as persist2,    # qT, q2T, stats
            tc.tile_pool(name="wstream", bufs=1) as wpool,
            tc.tile_pool(name="glu", bufs=2) as glu,
            tc.tile_pool(name="small", bufs=2) as small,
            tc.tile_pool(name="outp", bufs=1) as outp,
            tc.tile_pool(name="psum", bufs=8, space="PSUM") as psum,
        ):
            _consts = {}

            def const_ap(val):
                if val not in _consts:
                    ct = cpool.tile([P, 1], F32, tag=f"c{len(_consts)}",
                                    name=f"const_{len(_consts)}")
                    nc.vector.memset(ct, val)
                    _consts[val] = ct
                return _consts[val]

            for v in (1e-6, MAGIC, -MAGIC, -MAGIC * FIXI):
                const_ap(v)

            def alloc_pass_tiles():
                qT = persist2.tile([P, NH, pass_tt * P], BF16, tag="qT", bufs=2)
                q2T = persist2.tile([P, NS2, pass_tt * P], BF16, tag="q2T",
                                    bufs=1)
                z = persist.tile([P, pass_tt, I], F32, tag="z")
                c1s = persist2.tile([P, pass_tt], F32, tag="c1s")
                c2 = persist2.tile([P, pass_tt], F32, tag="c2")
                ssz = persist2.tile([P, pass_tt, NOC], F32, tag="ssz")
                mxz = persist2.tile([P, pass_tt, NOC], F32, tag="mxz")
                return qT, q2T, z, c1s, c2, ssz, mxz

            # ================= Phase A: stage-1 stats + quant ===============
            def emit_phase_a(p, tt, st):
                qT, _, _, c1s = st[0], st[1], st[2], st[3]
                row0 = (p * pass_tt + tt) * P
                xh = []
                for h in range(2):
                    xc = xin_pool.tile([P, 2 * CH], F32, tag="xc",
                                       name=f"xc_{p}_{tt}_{h}")
                    nc.sync.dma_start(
                        xc, x[row0:row0 + P, h * 2 * CH:(h + 1) * 2 * CH])
                    xh.append(xc)
                ssp = small.tile([P, NHC], F32, tag="ssp")
                mxp = small.tile([P, NHC], F32, tag="mxp")
                xsq = glu.tile([P, CH], F32, tag="qp", bufs=2)
                for c in range(NHC):
                    xs = xh[c // 2][:, (c % 2) * CH:(c % 2 + 1) * CH]
                    nc.scalar.activation(xsq, xs, AF.Square)
                    nc.vector.reduce_sum(ssp[:, c:c + 1], xsq, axis=AX.X)
                    nc.vector.reduce_max(mxp[:, c:c + 1], xs, axis=AX.X,
                                         apply_absolute_value=True)
                ss = small.tile([P, 1], F32, tag="ss")
                mx = small.tile([P, 1], F32, tag="mx")
                nc.vector.reduce_sum(ss, ssp, axis=AX.X)
                nc.vector.reduce_max(mx, mxp, axis=AX.X)
                sq = small.tile([P, 1], F32, tag="sq")
                nc.scalar.activation(sq, ss, AF.Sqrt, scale=1.0 / H,
                                     bias=const_ap(1e-6))
                r = small.tile([P, 1], F32, tag="r")
                nc.vector.reciprocal(r, sq)           # r = rsqrt(mean+eps)
                mn = small.tile([P, 1], F32, tag="mn")
                nc.vector.tensor_tensor(mn, mx, r, ALU.mult)
                nc.vector.tensor_scalar_max(mn, mn, 1e-5)   # m_norm
                im = small.tile([P, 1], F32, tag="im")
                nc.vector.reciprocal(im, mn)
                a1 = small.tile([P, 1], F32, tag="a1")
                nc.vector.scalar_tensor_tensor(a1, r, 127.0, im,
                                               ALU.mult, ALU.mult)
                nc.vector.tensor_scalar_mul(c1s[:, tt:tt + 1], mn, k_c1)

                # quant chunks into [128, 1024] halves; one transpose per half
                for half in range(2):
                    qnh = glu.tile([P, 2 * CH], BF16, tag="qnh", bufs=2,
                                   name=f"qnh_{p}_{tt}_{half}")
                    for cc in range(2):
                        c = half * 2 + cc
                        xs = xh[c // 2][:, (c % 2) * CH:(c % 2 + 1) * CH]
                        qp = glu.tile([P, CH], F32, tag="qp", bufs=2)
                        nc.gpsimd.tensor_scalar(qp, xs, a1, MAGIC,
                                                ALU.mult, ALU.add)
                        nc.scalar.activation(qnh[:, cc * CH:(cc + 1) * CH],
                                             qp, AF.Identity,
                                             bias=const_ap(-MAGIC))
                    # all transposes on one queue: concurrent XBAR transposes
                    # from both HWDGE queues corrupt each other
                    nc.sync.dma_start_transpose(
                        qT[:, half * (NH // 2):(half + 1) * (NH // 2),
                           tt * P:(tt + 1) * P],
                        qnh)

            # ============ Phase B: mm1 (psg+psy) + GLU ======================
            def emit_b_load(p, oc, wtiles, eng=None):
                # one big transfer: all 16 h-tiles of the (gate|y) chunk pair
                wg = wpool.tile([P, NH, 2, CH], FP8, tag="wg", bufs=3,
                                name=f"wg_{p}_{oc}")
                # SWDGE (Pool queue): stripes the 2MB load across all 16 SDMA
                # engines and keeps both HWDGE rings free for wd/out/transposes
                (eng or nc.gpsimd).dma_start(wg, wgp[oc])
                wtiles[oc] = wg

            def emit_b_unit(p, oc, tt, st, wtiles):
                qT, _, z, c1s, _, ssz, mxz = st
                tsl = slice(tt * P, (tt + 1) * P)
                wt = wtiles[oc]
                psg = psum.tile([P, CH], F32, tag="ps",
                                name=f"psg_{p}_{oc}_{tt}")
                psy = psum.tile([P, CH], F32, tag="ps",
                                name=f"psy_{p}_{oc}_{tt}")
                for s in range(NH):
                    nc.tensor.matmul(psg, qT[:, s, tsl], wt[:, s, 0],
                                     start=(s == 0), stop=(s == NH - 1))
                    nc.tensor.matmul(psy, qT[:, s, tsl], wt[:, s, 1],
                                     start=(s == 0), stop=(s == NH - 1))
                c1t = c1s[:, tt:tt + 1]
                # gp = gate_fx + MAGIC  (on ACT: DVE is the GLU bottleneck)
                gp = glu.tile([P, CH], F32, tag="gp")
                nc.scalar.activation(gp, psg, AF.Identity, scale=c1t,
                                     bias=const_ap(MAGIC))
                # sig = sigmoid(gate_fx / 2^16)
                sig = glu.tile([P, CH], F32, tag="sig")
                nc.scalar.activation(sig, gp, AF.Sigmoid,
                                     scale=FIXI, bias=const_ap(-MAGIC * FIXI))
                # sfx = rne(sig * 2^16)  (in place on sig)
                nc.scalar.activation(sig, sig, AF.Identity,
                                     scale=FIX, bias=const_ap(MAGIC))
                nc.scalar.activation(sig, sig, AF.Identity,
                                     bias=const_ap(-MAGIC))
                # prod = gate_fx * sig_fx
                prod = glu.tile([P, CH], F32, tag="prod")
                nc.vector.scalar_tensor_tensor(prod, gp, MAGIC, sig,
                                               ALU.subtract, ALU.mult)
                # swish_fx+MAGIC = rne(prod/2^16 - 0.5) + MAGIC
                nc.scalar.activation(prod, prod, AF.Identity,
                                     scale=FIXI, bias=const_ap(-0.5))
                nc.vector.tensor_scalar_add(prod, prod, MAGIC)
                # yfx (shares sig's buffers: sig is dead once prod is formed)
                yp = glu.tile([P, CH], F32, tag="sig")
                nc.vector.tensor_scalar(yp, psy, c1t, MAGIC, ALU.mult, ALU.add)
                nc.scalar.activation(yp, yp, AF.Identity, bias=const_ap(-MAGIC))
                # z_fx = floor(swish_fx * yfx / 2^16)
                nc.vector.scalar_tensor_tensor(prod, prod, MAGIC, yp,
                                               ALU.subtract, ALU.mult)
                nc.vector.tensor_scalar(prod, prod, FIXI, -0.5,
                                        ALU.mult, ALU.add)
                zsl = z[:, tt, oc * CH:(oc + 1) * CH]
                nc.vector.tensor_scalar(zsl, prod, MAGIC, MAGIC,
                                        ALU.add, ALU.subtract)
                # stage-2 stats (accumulated per chunk)
                zsq = glu.tile([P, CH], F32, tag="qp", bufs=2)
                nc.scalar.activation(zsq, zsl, AF.Square)
                nc.vector.reduce_sum(ssz[:, tt, oc:oc + 1], zsq, axis=AX.X)
                nc.vector.reduce_max(mxz[:, tt, oc:oc + 1], zsl,
                                     axis=AX.X, apply_absolute_value=True)

            # ====== Phase C: stage-2 scales + quant + transpose (one tt) ====
            def emit_phase_c(p, tt, st):
                _, q2T, z, _, c2, ssz, mxz = st
                ss2 = small.tile([P, 1], F32, tag="ss")
                mx2 = small.tile([P, 1], F32, tag="mx")
                nc.vector.reduce_sum(ss2, ssz[:, tt], axis=AX.X)
                nc.vector.reduce_max(mx2, mxz[:, tt], axis=AX.X)
                sq2 = small.tile([P, 1], F32, tag="sq")
                # mean(z^2) = ss2 * 2^-32 / I   (z = z_fx * 2^-16)
                nc.scalar.activation(sq2, ss2, AF.Sqrt,
                                     scale=FIXI * FIXI / I, bias=const_ap(1e-6))
                r2 = small.tile([P, 1], F32, tag="r")
                nc.vector.reciprocal(r2, sq2)
                mn2 = small.tile([P, 1], F32, tag="mn")
                nc.vector.tensor_scalar(mn2, mx2, FIXI, None, ALU.mult)
                nc.vector.tensor_tensor(mn2, mn2, r2, ALU.mult)
                nc.vector.tensor_scalar_max(mn2, mn2, 1e-5)
                im2 = small.tile([P, 1], F32, tag="im")
                nc.vector.reciprocal(im2, mn2)
                a2 = small.tile([P, 1], F32, tag="a1")
                # a2 = (r2 * 127 * 2^-16) * (1/mnorm2)   (applied to z_fx)
                nc.vector.scalar_tensor_tensor(a2, r2, 127.0 * FIXI, im2,
                                               ALU.mult, ALU.mult)
                nc.vector.tensor_scalar_mul(c2[:, tt:tt + 1], mn2, k_c2)

                # fine-grained quant+transpose: chunk 0 transposes alone so
                # mm2 can start immediately; remaining chunks go in pairs and
                # trickle in during D's early steps (step s2 only needs
                # chunk ceil(s2/4))
                tsl = slice(tt * P, (tt + 1) * P)
                pend = None     # (staging tile, first s2-tile, n s2-tiles)
                for c in range(NOC):
                    qp2 = glu.tile([P, CH], F32, tag="qp", bufs=2)
                    # GPSIMD is idle; keep DVE free for the tail GLU chains
                    nc.gpsimd.tensor_scalar(qp2, z[:, tt, c * CH:(c + 1) * CH],
                                            a2, MAGIC, ALU.mult, ALU.add)
                    if c == 0:
                        q2s = glu.tile([P, CH], BF16, tag="q2s0", bufs=2,
                                       name=f"q2s_{p}_{tt}_{c}")
                        nc.scalar.activation(q2s, qp2, AF.Identity,
                                             bias=const_ap(-MAGIC))
                        nc.sync.dma_start_transpose(q2T[:, 0:4, tsl], q2s)
                    else:
                        half = (c - 1) % 2
                        if half == 0:
                            q2s = glu.tile([P, 2 * CH], BF16, tag="q2s",
                                           bufs=3, name=f"q2s_{p}_{tt}_{c}")
                        nc.scalar.activation(q2s[:, half * CH:(half + 1) * CH],
                                             qp2, AF.Identity,
                                             bias=const_ap(-MAGIC))
                        if half == 1:
                            s20 = 4 + (c - 2) * 4
                            nc.sync.dma_start_transpose(
                                q2T[:, s20:s20 + 8, tsl], q2s)

            # ================= Phase D: mm2 + output ========================
            def load_wd(p, s2, wd_tiles):
                # wd exclusively on the scalar queue: the sync queue carries
                # the latency-critical C/A transposes during this window
                wd = wpool.tile([P, NOT * CH], FP8, tag="wd", bufs=WD_BUFS,
                                name=f"wd_{p}_{s2}")
                nc.scalar.dma_start(wd, wdt[s2 * P:(s2 + 1) * P, :])
                wd_tiles[s2] = wd

            def emit_phase_d(p, st, wd_tiles, wtiles_next):
                _, q2T, _, _, c2, _, _ = st

                ps = {}
                for tt in range(pass_tt):
                    for ot in range(NOT):
                        ps[(tt, ot)] = psum.tile(
                            [P, CH], F32, tag="ps", name=f"psd_{p}_{tt}_{ot}")

                def emit_step(tt, s2):
                    tsl = slice(tt * P, (tt + 1) * P)
                    wd = wd_tiles[s2]
                    for ot in range(NOT):
                        nc.tensor.matmul(ps[(tt, ot)], q2T[:, s2, tsl],
                                         wd[:, ot * CH:(ot + 1) * CH],
                                         start=(s2 == 0), stop=(s2 == NS2 - 1))
                    if s2 + PF_D < NS2 and tt == 0:
                        load_wd(p, s2 + PF_D, wd_tiles)
                    # prefetch next pass's first wg chunks on the (idle) sync
                    # ring so B(p+1) starts right after D(p)'s last matmul
                    if tt == 0 and wtiles_next is not None and s2 in (30, 34):
                        emit_b_load(p + 1, (s2 - 30) // 4, wtiles_next)

                def drain(tt):
                    row0 = (p * pass_tt + tt) * P
                    for ot in range(NOT):
                        ot_t = outp.tile([P, CH], F32, tag="oc", bufs=2,
                                         name=f"od_{p}_{tt}_{ot}")
                        nc.scalar.activation(ot_t, ps[(tt, ot)], AF.Copy,
                                             scale=c2[:, tt:tt + 1])
                        nc.scalar.dma_start(
                            out[row0:row0 + P, ot * CH:(ot + 1) * CH], ot_t)

                for step in range(NS2 + LAG_D):
                    if step < NS2:
                        emit_step(0, step)
                    if step >= LAG_D and step - LAG_D < NS2:
                        emit_step(1, step - LAG_D)
                    if step == NS2 - 1:
                        drain(0)
                drain(1)

            # =========================== driver =============================
            st = None
            st_next = None
            wtiles_next = None
            for p in range(NPASS):
                if p == 0:
                    st = alloc_pass_tiles()
                    for tt in range(pass_tt):
                        emit_phase_a(p, tt, st)
                else:
                    st = st_next
                # B with a 3-unit tail reorder: the last three tt0 units run
                # back-to-back so their GLUs + C0's serial chain complete
                # under the three tt1 tail units' matmuls (needs wg bufs=3):
                #  (0,t0),(0,t1),...,(7,t0),(7,t1),(8,t0),(9,t0),(10,t0),
                #  [C t0], (8,t1),(9,t1),(10,t1), [C t1 + A(p+1)]
                wtiles = wtiles_next if wtiles_next is not None else {}
                wd_tiles = {}
                if p == 0:
                    emit_b_load(p, 0, wtiles)
                    emit_b_load(p, 1, wtiles)
                units = []
                for oc in range(NOC):
                    units.append((oc, 0))
                    if oc >= 1:
                        units.append((oc - 1, 1))
                units += ["C0", (NOC - 1, 1), "C1"]
                for u in units:
                    if u == "C0":
                        # hoist phase-D weight prefetch ahead of the C/A
                        # transposes so the first D chains start immediately
                        for s2 in range(min(PF_D, NS2)):
                            load_wd(p, s2, wd_tiles)
                        emit_phase_c(p, 0, st)
                    elif u == "C1":
                        emit_phase_c(p, 1, st)
                        if p + 1 < NPASS:
                            st_next = alloc_pass_tiles()
                            for tt in range(pass_tt):
                                emit_phase_a(p + 1, tt, st_next)
                    else:
                        oc, tt = u
                        if oc + 2 < NOC and tt == 0:
                            emit_b_load(p, oc + 2, wtiles)
                        emit_b_unit(p, oc, tt, st, wtiles)
                wtiles_next = {} if p + 1 < NPASS else None
                emit_phase_d(p, st, wd_tiles, wtiles_next)
    nc.compile()
    return nc


def _quant_weights(w):
    """Reference weight_quant, computed with jax on CPU to match the oracle
    bit-exactly (numpy's f32 mean differs in the last ulp, which flips
    round(w*s) for weights landing exactly on the +-0.5 boundary)."""
    import jax
    import jax.numpy as jnp
    cpu = jax.devices("cpu")[0]
    with jax.default_device(cpu):
        wj = jnp.asarray(np.asarray(w, dtype=np.float32))
        s = 1.0 / jnp.maximum(jnp.mean(jnp.abs(wj)), 1e-5)
        tern = jnp.clip(jnp.round(wj * s), -1.0, 1.0)
        tern_np = np.asarray(jax.device_get(tern), dtype=np.float32)
        deq = float(jax.device_get(1.0 / s))
    return tern_np, deq


def kernel(x, w_gate, w_down):
    B, S, H = x.shape            # (4, 2048, 2048)
    I = w_down.shape[1]          # 5632
    N_CORES = 8
    T_TOTAL = B * S
    T = T_TOTAL // N_CORES
    NOC = I // CH
    NH = H // P

    Xf = np.ascontiguousarray(np.asarray(x, dtype=np.float32).reshape(T_TOTAL, H))
    g_tern, wg_deq = _quant_weights(w_gate)
    d_tern, wd_deq = _quant_weights(w_down)

    nc = build_program(T, H, I, wg_deq, wd_deq)

    # pack gate|y chunk pairs: [NOC, p, s, 2, CH] from g_tern [2I, H]
    gT = np.ascontiguousarray(g_tern.T).astype(ml_dtypes.float8_e4m3)  # [H, 2I]
    gate_part = gT[:, :I].reshape(H, NOC, 1, CH)
    y_part = gT[:, I:].reshape(H, NOC, 1, CH)
    wgp = np.concatenate([gate_part, y_part], axis=2)    # [H, NOC, 2, CH]
    wgp = np.ascontiguousarray(
        wgp.reshape(NH, P, NOC, 2, CH).transpose(2, 1, 0, 3, 4))
    wdt = np.ascontiguousarray(d_tern.T).astype(ml_dtypes.float8_e4m3)  # [I, H]

    in_maps = []
    for c in range(N_CORES):
        in_maps.append({
            "x": np.ascontiguousarray(Xf[c * T:(c + 1) * T]),
            "wgp": wgp,
            "wdt": wdt,
        })
    res = run_bass_kernel_spmd(nc, in_maps, core_ids=list(range(N_CORES)))
    global LAST_RESULTS
    LAST_RESULTS = res
    outs = [res.results[c]["out"] for c in range(N_CORES)]
    full = np.concatenate(outs, axis=0).reshape(B, S, H).astype(np.float32)
    return full


LAST_RESULTS = None


if __name__ == "__main__":
    rng = np.random.default_rng(0)
    x = rng.standard_normal((4, 2048, 2048), dtype=np.float32)
    wg = (rng.standard_normal((11264, 2048), dtype=np.float32) * 0.02)
    wd = (rng.standard_normal((2048, 5632), dtype=np.float32) * 0.02)
    y = kernel(x, wg, wd)
    print(y.shape, y.dtype, np.abs(y).max())


# revision 28
# speedup vs baseline: 1.0368x; 1.0368x over previous
"""HGRNBitMLPFixed (BitLinear MLP with Q16 fixed-point GLU) on 8 Trainium2 cores.

Strategy (v3)
-------------
Data-parallel over tokens: 8192 tokens -> 1024 per core, weights replicated.
Host pre-quantizes weights (ternary, exact in fp8e4m3; the quant scale is
computed with jax to match the oracle's f32 mean bit-exactly) and packs them;
the device does everything else:

  per token-tile [128, H]:
    stats (sum x^2, absmax) -> r = 1/sqrt(mean+eps), quant scale a1, dequant c1
    q = rne(x * a1)  (int8-valued, exact in bf16)          [natural layout]
    DMA-XBAR transposes -> qT [h, t]
  mm1: psum[t, o] = sum_h qT[h,t]^T wgT[h,o]   (exact integer sums in fp32 PSUM)
  GLU in Q16 fixed point (all exact fp32 integer arithmetic; rne via the
  +/- 1.5*2^23 magic-number trick, floor via rne(x-0.5)):
    gfx = rne(psum_g * c1 * 2^16); sig = sigmoid(gfx/2^16)
    sfx = rne(sig * 2^16); swish = floor(gfx*sfx / 2^16)
    yfx = rne(psum_y * c1 * 2^16); z_fx = floor(swish*yfx / 2^16)
  stage-2 stats over z_fx -> a2, c2; q2 = rne(z_fx * a2); transpose -> q2T
  mm2: psum[t, o] = sum_i q2T^T wdT; out = psum * c2

v3 performance changes vs v2 (identical arithmetic):
  - batched DMA: all 16 h-tiles of one mm1 weight chunk come in ONE
    dma_start (weights repacked in DRAM as [128, 16, NOC, 2, CH]); x loads,
    out stores and XBAR transposes are batched too.  Each dma_start costs
    ~600ns of HWDGE queue time regardless of size, and one large InstDMACopy
    is split across all 16 SDMA engines, so fewer+bigger is strictly better.
  - per pass: 11 wg loads (was 176), 8 transposes (was 30), 4 x loads,
    2 out stores.
  - all transposes on the sync queue (concurrent XBAR transposes from both
    HWDGE queues corrupt each other); big weight loads on the scalar queue.
  - LAG_D 20 -> 14 so wd streaming needs 18 bufs instead of 22 (SBUF).
"""

import numpy as np
import ml_dtypes

import concourse.bass as bass
import concourse.mybir as mybir
import concourse.tile as tile
from concourse import bacc
from concourse.bass_utils import run_bass_kernel_spmd

F32 = mybir.dt.float32
BF16 = mybir.dt.bfloat16
FP8 = mybir.dt.float8e4
AF = mybir.ActivationFunctionType
ALU = mybir.AluOpType
AX = mybir.AxisListType

P = 128          # partitions
CH = 512         # free-dim chunk (one PSUM bank of fp32)
MAGIC = 12582912.0   # 1.5 * 2^23: x + MAGIC - MAGIC == rne(x) for |x| < 2^22
FIX = 65536.0        # 2^16 (Q16)
FIXI = 1.0 / 65536.0
LAG_D = 10       # tt1 chain lag (in s2 steps) in phase D
PF_D = 2         # wd prefetch distance (s2 steps)
WD_BUFS = 17     # > LAG_D + PF_D + 1 (alive span); extra bufs = ring slack


def build_program(T, H, I, wg_deq, wd_deq):
    pass_tt = 2
    NT = T // P               # token tiles (8)
    NH = H // P               # h-tiles (contraction of mm1) (16)
    NHC = H // CH             # h chunks of 512 (4)
    NOC = I // CH             # inter chunks of 512 (11)
    NS2 = I // P              # i-tiles (contraction of mm2) (44)
    NOT = H // CH             # output chunks of 512 (4)
    assert NT % pass_tt == 0
    NPASS = NT // pass_tt
    # C-phase transpose split: s2-tiles [0,20) and [20,44)
    CSPLIT = 20

    nc = bacc.Bacc(None, target_bir_lowering=False)
    x = nc.dram_tensor("x", [T, H], F32, kind="ExternalInput").ap()
    # packed gate|y chunk pairs: [NOC, p, s, 2, CH] fp8 (p=row within h-tile)
    # -> one oc load is 128 contiguous 16KB runs (one DMA descriptor per
    #    partition) instead of 2048 scattered 1KB runs
    wgp = nc.dram_tensor("wgp", [NOC, P, NH, 2, CH], FP8,
                         kind="ExternalInput").ap()
    wdt = nc.dram_tensor("wdt", [I, H], FP8, kind="ExternalInput").ap()
    out = nc.dram_tensor("out", [T, H], F32, kind="ExternalOutput").ap()

    k_c1 = float(wg_deq) * FIX / 127.0   # c1s = mnorm * k_c1 (includes 2^16)
    k_c2 = float(wd_deq) / 127.0         # c2 = mnorm2 * k_c2

    with tile.TileContext(nc) as tc:
        with (
            tc.tile_pool(name="consts", bufs=1) as cpool,
            tc.tile_pool(name="xin", bufs=2) as xin_pool,
            tc.tile_pool(name="persist", bufs=1) as persist,      # z
            tc.tile_pool(name="persist2", bufs=2) as persist2,    # qT, q2T, stats
            tc.tile_pool(name="wstream", bufs=1) as wpool,
            tc.tile_pool(name="glu", bufs=2) as glu,
            tc.tile_pool(name="small", bufs=2) as small,
            tc.tile_pool(name="outp", bufs=1) as outp,
            tc.tile_pool(name="psum", bufs=8, space="PSUM") as psum,
        ):
            _consts = {}

            def const_ap(val):
                if val not in _consts:
                    ct = cpool.tile([P, 1], F32, tag=f"c{len(_consts)}",
                                    name=f"const_{len(_consts)}")
                    nc.vector.memset(ct, val)
                    _consts[val] = ct
                return _consts[val]

            for v in (1e-6, MAGIC, -MAGIC, -MAGIC * FIXI):
                const_ap(v)

            def alloc_pass_tiles():
                qT = persist2.tile([P, NH, pass_tt * P], BF16, tag="qT", bufs=2)
                q2T = persist2.tile([P, NS2, pass_tt * P], BF16, tag="q2T",
                                    bufs=1)
                z = persist.tile([P, pass_tt, I], F32, tag="z")
                c1s = persist2.tile([P, pass_tt], F32, tag="c1s")
                c2 = persist2.tile([P, pass_tt], F32, tag="c2")
                ssz = persist2.tile([P, pass_tt, NOC], F32, tag="ssz")
                mxz = persist2.tile([P, pass_tt, NOC], F32, tag="mxz")
                ssA = persist2.tile([P, pass_tt], F32, tag="ssA")
                mxA = persist2.tile([P, pass_tt], F32, tag="mxA")
                return qT, q2T, z, c1s, c2, ssz, mxz, ssA, mxA

            # ================= Phase A: stage-1 stats + quant ===============
            def emit_phase_a(p, tt, st):
                qT, _, _, c1s = st[0], st[1], st[2], st[3]
                row0 = (p * pass_tt + tt) * P
                xh = []
                for h in range(2):
                    xc = xin_pool.tile([P, 2 * CH], F32, tag="xc",
                                       name=f"xc_{p}_{tt}_{h}")
                    nc.sync.dma_start(
                        xc, x[row0:row0 + P, h * 2 * CH:(h + 1) * 2 * CH])
                    xh.append(xc)
                ssp = small.tile([P, NHC], F32, tag="ssp")
                mxp = small.tile([P, NHC], F32, tag="mxp")
                xsq = glu.tile([P, CH], F32, tag="qp", bufs=2)
                for c in range(NHC):
                    xs = xh[c // 2][:, (c % 2) * CH:(c % 2 + 1) * CH]
                    nc.scalar.activation(xsq, xs, AF.Square)
                    nc.vector.reduce_sum(ssp[:, c:c + 1], xsq, axis=AX.X)
                    nc.vector.reduce_max(mxp[:, c:c + 1], xs, axis=AX.X,
                                         apply_absolute_value=True)
                ss = small.tile([P, 1], F32, tag="ss")
                mx = small.tile([P, 1], F32, tag="mx")
                nc.vector.reduce_sum(ss, ssp, axis=AX.X)
                nc.vector.reduce_max(mx, mxp, axis=AX.X)
                sq = small.tile([P, 1], F32, tag="sq")
                nc.scalar.activation(sq, ss, AF.Sqrt, scale=1.0 / H,
                                     bias=const_ap(1e-6))
                r = small.tile([P, 1], F32, tag="r")
                nc.vector.reciprocal(r, sq)           # r = rsqrt(mean+eps)
                mn = small.tile([P, 1], F32, tag="mn")
                nc.vector.tensor_tensor(mn, mx, r, ALU.mult)
                nc.vector.tensor_scalar_max(mn, mn, 1e-5)   # m_norm
                im = small.tile([P, 1], F32, tag="im")
                nc.vector.reciprocal(im, mn)
                a1 = small.tile([P, 1], F32, tag="a1")
                nc.vector.scalar_tensor_tensor(a1, r, 127.0, im,
                                               ALU.mult, ALU.mult)
                nc.vector.tensor_scalar_mul(c1s[:, tt:tt + 1], mn, k_c1)

                # quant chunks into [128, 1024] halves; one transpose per half
                for half in range(2):
                    qnh = glu.tile([P, 2 * CH], BF16, tag="qnh", bufs=2,
                                   name=f"qnh_{p}_{tt}_{half}")
                    for cc in range(2):
                        c = half * 2 + cc
                        xs = xh[c // 2][:, (c % 2) * CH:(c % 2 + 1) * CH]
                        qp = glu.tile([P, CH], F32, tag="qp", bufs=2)
                        nc.vector.tensor_scalar(qp, xs, a1, MAGIC,
                                                ALU.mult, ALU.add)
                        nc.scalar.activation(qnh[:, cc * CH:(cc + 1) * CH],
                                             qp, AF.Identity,
                                             bias=const_ap(-MAGIC))
                    # all transposes on one queue: concurrent XBAR transposes
                    # from both HWDGE queues corrupt each other
                    nc.sync.dma_start_transpose(
                        qT[:, half * (NH // 2):(half + 1) * (NH // 2),
                           tt * P:(tt + 1) * P],
                        qnh)

            # ============ Phase B: mm1 (psg+psy) + GLU ======================
            def emit_b_load(p, oc, wtiles, eng=None):
                # one big transfer: all 16 h-tiles of the (gate|y) chunk pair
                wg = wpool.tile([P, NH, 2, CH], FP8, tag="wg", bufs=3,
                                name=f"wg_{p}_{oc}")
                # SWDGE (Pool queue): stripes the 2MB load across all 16 SDMA
                # engines and keeps both HWDGE rings free for wd/out/transposes
                (eng or nc.gpsimd).dma_start(wg, wgp[oc])
                wtiles[oc] = wg

            def emit_b_unit(p, oc, tt, st, wtiles):
                qT, _, z, c1s, _, ssz, mxz = st[:7]
                tsl = slice(tt * P, (tt + 1) * P)
                wt = wtiles[oc]
                psg = psum.tile([P, CH], F32, tag="ps",
                                name=f"psg_{p}_{oc}_{tt}")
                psy = psum.tile([P, CH], F32, tag="ps",
                                name=f"psy_{p}_{oc}_{tt}")
                for s in range(NH):
                    nc.tensor.matmul(psg, qT[:, s, tsl], wt[:, s, 0],
                                     start=(s == 0), stop=(s == NH - 1))
                    nc.tensor.matmul(psy, qT[:, s, tsl], wt[:, s, 1],
                                     start=(s == 0), stop=(s == NH - 1))
                c1t = c1s[:, tt:tt + 1]
                # gp = gate_fx + MAGIC  (on ACT: DVE is the GLU bottleneck)
                gp = glu.tile([P, CH], F32, tag="gp")
                nc.scalar.activation(gp, psg, AF.Identity, scale=c1t,
                                     bias=const_ap(MAGIC))
                # sig = sigmoid(gate_fx / 2^16)
                sig = glu.tile([P, CH], F32, tag="sig")
                nc.scalar.activation(sig, gp, AF.Sigmoid,
                                     scale=FIXI, bias=const_ap(-MAGIC * FIXI))
                # sfx = rne(sig * 2^16)  (in place on sig)
                nc.scalar.activation(sig, sig, AF.Identity,
                                     scale=FIX, bias=const_ap(MAGIC))
                nc.scalar.activation(sig, sig, AF.Identity,
                                     bias=const_ap(-MAGIC))
                # prod = gate_fx * sig_fx
                prod = glu.tile([P, CH], F32, tag="prod")
                nc.vector.scalar_tensor_tensor(prod, gp, MAGIC, sig,
                                               ALU.subtract, ALU.mult)
                # swish_fx+MAGIC = rne(prod/2^16 - 0.5) + MAGIC
                nc.scalar.activation(prod, prod, AF.Identity,
                                     scale=FIXI, bias=const_ap(-0.5))
                nc.vector.tensor_scalar_add(prod, prod, MAGIC)
                # yfx (shares sig's buffers: sig is dead once prod is formed)
                yp = glu.tile([P, CH], F32, tag="sig")
                nc.vector.tensor_scalar(yp, psy, c1t, MAGIC, ALU.mult, ALU.add)
                nc.scalar.activation(yp, yp, AF.Identity, bias=const_ap(-MAGIC))
                # z_fx = floor(swish_fx * yfx / 2^16)
                nc.vector.scalar_tensor_tensor(prod, prod, MAGIC, yp,
                                               ALU.subtract, ALU.mult)
                nc.vector.tensor_scalar(prod, prod, FIXI, -0.5,
                                        ALU.mult, ALU.add)
                zsl = z[:, tt, oc * CH:(oc + 1) * CH]
                nc.vector.tensor_scalar(zsl, prod, MAGIC, MAGIC,
                                        ALU.add, ALU.subtract)
                # stage-2 stats (accumulated per chunk)
                zsq = glu.tile([P, CH], F32, tag="qp", bufs=2)
                nc.scalar.activation(zsq, zsl, AF.Square)
                nc.vector.reduce_sum(ssz[:, tt, oc:oc + 1], zsq, axis=AX.X)
                nc.vector.reduce_max(mxz[:, tt, oc:oc + 1], zsl,
                                     axis=AX.X, apply_absolute_value=True)
                if oc == NOC - 2:
                    # pre-reduce chunks 0..NOC-2 off the C critical path
                    ssA, mxA = st[7], st[8]
                    nc.vector.reduce_sum(ssA[:, tt:tt + 1],
                                         ssz[:, tt, :NOC - 1], axis=AX.X)
                    nc.vector.reduce_max(mxA[:, tt:tt + 1],
                                         mxz[:, tt, :NOC - 1], axis=AX.X)

            # ====== Phase C: stage-2 scales + quant + transpose (one tt) ====
            def emit_phase_c(p, tt, st):
                _, q2T, z, _, c2, ssz, mxz, ssA, mxA = st
                ss2 = small.tile([P, 1], F32, tag="ss")
                mx2 = small.tile([P, 1], F32, tag="mx")
                nc.vector.tensor_tensor(ss2, ssA[:, tt:tt + 1],
                                        ssz[:, tt, NOC - 1:NOC], ALU.add)
                nc.vector.tensor_tensor(mx2, mxA[:, tt:tt + 1],
                                        mxz[:, tt, NOC - 1:NOC], ALU.max)
                sq2 = small.tile([P, 1], F32, tag="sq")
                # mean(z^2) = ss2 * 2^-32 / I   (z = z_fx * 2^-16)
                nc.scalar.activation(sq2, ss2, AF.Sqrt,
                                     scale=FIXI * FIXI / I, bias=const_ap(1e-6))
                r2 = small.tile([P, 1], F32, tag="r")
                nc.vector.reciprocal(r2, sq2)
                mn2 = small.tile([P, 1], F32, tag="mn")
                nc.vector.tensor_scalar(mn2, mx2, FIXI, None, ALU.mult)
                nc.vector.tensor_tensor(mn2, mn2, r2, ALU.mult)
                nc.vector.tensor_scalar_max(mn2, mn2, 1e-5)
                im2 = small.tile([P, 1], F32, tag="im")
                nc.vector.reciprocal(im2, mn2)
                a2 = small.tile([P, 1], F32, tag="a1")
                # a2 = (r2 * 127 * 2^-16) * (1/mnorm2)   (applied to z_fx)
                nc.vector.scalar_tensor_tensor(a2, r2, 127.0 * FIXI, im2,
                                               ALU.mult, ALU.mult)
                nc.vector.tensor_scalar_mul(c2[:, tt:tt + 1], mn2, k_c2)

                # fine-grained quant+transpose: chunk 0 transposes alone so
                # mm2 can start immediately; remaining chunks go in pairs and
                # trickle in during D's early steps (step s2 only needs
                # chunk ceil(s2/4))
                tsl = slice(tt * P, (tt + 1) * P)
                pend = None     # (staging tile, first s2-tile, n s2-tiles)
                for c in range(NOC):
                    qp2 = glu.tile([P, CH], F32, tag="qp", bufs=2)
                    nc.vector.tensor_scalar(qp2, z[:, tt, c * CH:(c + 1) * CH],
                                            a2, MAGIC, ALU.mult, ALU.add)
                    if c == 0:
                        q2s = glu.tile([P, CH], BF16, tag="q2s0", bufs=2,
                                       name=f"q2s_{p}_{tt}_{c}")
                        nc.scalar.activation(q2s, qp2, AF.Identity,
                                             bias=const_ap(-MAGIC))
                        nc.sync.dma_start_transpose(q2T[:, 0:4, tsl], q2s)
                    else:
                        half = (c - 1) % 2
                        if half == 0:
                            q2s = glu.tile([P, 2 * CH], BF16, tag="q2s",
                                           bufs=3, name=f"q2s_{p}_{tt}_{c}")
                        nc.scalar.activation(q2s[:, half * CH:(half + 1) * CH],
                                             qp2, AF.Identity,
                                             bias=const_ap(-MAGIC))
                        if half == 1:
                            s20 = 4 + (c - 2) * 4
                            nc.sync.dma_start_transpose(
                                q2T[:, s20:s20 + 8, tsl], q2s)

            # ================= Phase D: mm2 + output ========================
            def load_wd(p, s2, wd_tiles):
                # wd exclusively on the scalar queue: the sync queue carries
                # the latency-critical C/A transposes during this window
                wd = wpool.tile([P, NOT * CH], FP8, tag="wd", bufs=WD_BUFS,
                                name=f"wd_{p}_{s2}")
                nc.scalar.dma_start(wd, wdt[s2 * P:(s2 + 1) * P, :])
                wd_tiles[s2] = wd

            def emit_phase_d(p, st, wd_tiles, wtiles_next):
                _, q2T, _, _, c2 = st[:5]

                ps = {}
                for tt in range(pass_tt):
                    for ot in range(NOT):
                        ps[(tt, ot)] = psum.tile(
                            [P, CH], F32, tag="ps", name=f"psd_{p}_{tt}_{ot}")

                def emit_step(tt, s2):
                    tsl = slice(tt * P, (tt + 1) * P)
                    wd = wd_tiles[s2]
                    for ot in range(NOT):
                        nc.tensor.matmul(ps[(tt, ot)], q2T[:, s2, tsl],
                                         wd[:, ot * CH:(ot + 1) * CH],
                                         start=(s2 == 0), stop=(s2 == NS2 - 1))
                    if s2 + PF_D < NS2 and tt == 0:
                        load_wd(p, s2 + PF_D, wd_tiles)
                    # prefetch next pass's first wg chunks on the (idle) sync
                    # ring so B(p+1) starts right after D(p)'s last matmul
                    if tt == 0 and wtiles_next is not None and s2 in (30, 34):
                        emit_b_load(p + 1, (s2 - 30) // 4, wtiles_next)

                def drain(tt):
                    row0 = (p * pass_tt + tt) * P
                    for ot in range(NOT):
                        ot_t = outp.tile([P, CH], F32, tag="oc", bufs=2,
                                         name=f"od_{p}_{tt}_{ot}")
                        nc.scalar.activation(ot_t, ps[(tt, ot)], AF.Copy,
                                             scale=c2[:, tt:tt + 1])
                        nc.scalar.dma_start(
                            out[row0:row0 + P, ot * CH:(ot + 1) * CH], ot_t)

                for step in range(NS2 + LAG_D):
                    if step < NS2:
                        emit_step(0, step)
                    if step >= LAG_D and step - LAG_D < NS2:
                        emit_step(1, step - LAG_D)
                    if step == NS2 - 1:
                        drain(0)
                drain(1)

            # =========================== driver =============================
            st = None
            st_next = None
            wtiles_next = None
            for p in range(NPASS):
                if p == 0:
                    st = alloc_pass_tiles()
                    for tt in range(pass_tt):
                        emit_phase_a(p, tt, st)
                else:
                    st = st_next
                # B with a 3-unit tail reorder: the last three tt0 units run
                # back-to-back so their GLUs + C0's serial chain complete
                # under the three tt1 tail units' matmuls (needs wg bufs=3):
                #  (0,t0),(0,t1),...,(7,t0),(7,t1),(8,t0),(9,t0),(10,t0),
                #  [C t0], (8,t1),(9,t1),(10,t1), [C t1 + A(p+1)]
                wtiles = wtiles_next if wtiles_next is not None else {}
                wd_tiles = {}
                if p == 0:
                    emit_b_load(p, 0, wtiles)
                    emit_b_load(p, 1, wtiles)
                units = []
                for oc in range(NOC):
                    units.append((oc, 0))
                    if oc >= 1:
                        units.append((oc - 1, 1))
                units += ["C0", (NOC - 1, 1), "C1"]
                for u in units:
                    if u == "C0":
                        # hoist phase-D weight prefetch ahead of the C/A
                        # transposes so the first D chains start immediately
                        for s2 in range(min(PF_D, NS2)):
                            load_wd(p, s2, wd_tiles)
                        emit_phase_c(p, 0, st)
                    elif u == "C1":
                        emit_phase_c(p, 1, st)
                        if p + 1 < NPASS:
                            st_next = alloc_pass_tiles()
                            for tt in range(pass_tt):
                                emit_phase_a(p + 1, tt, st_next)
                    else:
                        oc, tt = u
                        if oc + 2 < NOC and tt == 0:
                            emit_b_load(p, oc + 2, wtiles)
                        emit_b_unit(p, oc, tt, st, wtiles)
                wtiles_next = {} if p + 1 < NPASS else None
                emit_phase_d(p, st, wd_tiles, wtiles_next)
    nc.compile()
    return nc


def _quant_weights(w):
    """Reference weight_quant, computed with jax on CPU to match the oracle
    bit-exactly (numpy's f32 mean differs in the last ulp, which flips
    round(w*s) for weights landing exactly on the +-0.5 boundary)."""
    import jax
    import jax.numpy as jnp
    cpu = jax.devices("cpu")[0]
    with jax.default_device(cpu):
        wj = jnp.asarray(np.asarray(w, dtype=np.float32))
        s = 1.0 / jnp.maximum(jnp.mean(jnp.abs(wj)), 1e-5)
        tern = jnp.clip(jnp.round(wj * s), -1.0, 1.0)
        tern_np = np.asarray(jax.device_get(tern), dtype=np.float32)
        deq = float(jax.device_get(1.0 / s))
    return tern_np, deq


def kernel(x, w_gate, w_down):
    B, S, H = x.shape            # (4, 2048, 2048)
    I = w_down.shape[1]          # 5632
    N_CORES = 8
    T_TOTAL = B * S
    T = T_TOTAL // N_CORES
    NOC = I // CH
    NH = H // P

    Xf = np.ascontiguousarray(np.asarray(x, dtype=np.float32).reshape(T_TOTAL, H))
    g_tern, wg_deq = _quant_weights(w_gate)
    d_tern, wd_deq = _quant_weights(w_down)

    nc = build_program(T, H, I, wg_deq, wd_deq)

    # pack gate|y chunk pairs: [NOC, p, s, 2, CH] from g_tern [2I, H]
    gT = np.ascontiguousarray(g_tern.T).astype(ml_dtypes.float8_e4m3)  # [H, 2I]
    gate_part = gT[:, :I].reshape(H, NOC, 1, CH)
    y_part = gT[:, I:].reshape(H, NOC, 1, CH)
    wgp = np.concatenate([gate_part, y_part], axis=2)    # [H, NOC, 2, CH]
    wgp = np.ascontiguousarray(
        wgp.reshape(NH, P, NOC, 2, CH).transpose(2, 1, 0, 3, 4))
    wdt = np.ascontiguousarray(d_tern.T).astype(ml_dtypes.float8_e4m3)  # [I, H]

    in_maps = []
    for c in range(N_CORES):
        in_maps.append({
            "x": np.ascontiguousarray(Xf[c * T:(c + 1) * T]),
            "wgp": wgp,
            "wdt": wdt,
        })
    res = run_bass_kernel_spmd(nc, in_maps, core_ids=list(range(N_CORES)))
    global LAST_RESULTS
    LAST_RESULTS = res
    outs = [res.results[c]["out"] for c in range(N_CORES)]
    full = np.concatenate(outs, axis=0).reshape(B, S, H).astype(np.float32)
    return full


LAST_RESULTS = None


if __name__ == "__main__":
    rng = np.random.default_rng(0)
    x = rng.standard_normal((4, 2048, 2048), dtype=np.float32)
    wg = (rng.standard_normal((11264, 2048), dtype=np.float32) * 0.02)
    wd = (rng.standard_normal((2048, 5632), dtype=np.float32) * 0.02)
    y = kernel(x, wg, wd)
    print(y.shape, y.dtype, np.abs(y).max())
